# revision 19
# baseline (speedup 1.0000x reference)
"""Trainium2 Bass kernel for BidirectionalCrossModalAttention (seq_len=1).

Math: with a single key, softmax == 1 exactly, so each MHA block reduces to
    mha(q, kv) = kv @ (w_out @ w_v).T + (w_out @ b_v + b_out)
i.e. one 1024x1024 matmul.  Gate matmuls partially fold into the same form;
the vision gate's v2t term folds onto text:  Gb1 @ v2t = (Gb1 @ W1) @ text + c.
Total device work: 12 x [1024x1024] matmuls per row + 4 sigmoid gates +
4 layernorms.

Precision split (driven by the 2e-2 gate):
  - 6 "main" matmuls (w0..w5, outputs feed residual sums directly): fp16
    weights x fp16 activations, normal PE mode.
  - 6 "gate" matmuls (sigmoid args, error damped by sigmoid slope) +
    LN-stats ones-matmuls: fp8e4 with DoubleRow (2x contraction per pass).
    fp8 weights are scaled 8x, fp8 activations 1/8x, so PSUM is unscaled.
  - Elementwise chain + residual streams: fp16. LN stats via PE ones-matmuls
    on fp8 copies; rstd via reciprocal_approx_fast (DVE).

Engine split: PE matmuls; DVE evictions (bias add / sum) + y-chain + stats;
ACT only Sigmoid + Sqrt (avoids activation-table thrash); GpSimd makes the
fp8 stat copies (yb, y^2) and fp8 downscale copies.

Layout: transposed — activations [feature(->128 partitions, 8 chunks),
rows(->free)].  Row blocks of 512 (one full PSUM bank per matmul output).
LayerNorm row-stats broadcast across partitions via a DMA bounce thru DRAM.

Sharding: pure data parallelism — batch 32768 = 8 x 4096 across cores.
"""

import numpy as np
import ml_dtypes

import concourse.bass as bass
import concourse.tile as tile
from concourse import bacc, mybir
from concourse import bass_utils
from concourse.bass import ts

F32 = mybir.dt.float32
F16 = mybir.dt.float16
F8 = mybir.dt.float8e4
AF = mybir.ActivationFunctionType
OP = mybir.AluOpType
PM = mybir.MatmulPerfMode

F8NP = ml_dtypes.float8_e4m3

DIM = 1024
BATCH = 32768
NCORES = 8
R = BATCH // NCORES      # rows per core
C = DIM // 128           # feature chunks
C2 = C // 2              # feature chunk pairs (DoubleRow)
EPS = 1e-5

NB = 512                 # rows per block (one PSUM bank of fp32)
NBLK = R // NB

WSCALE = 8.0             # fp8 weight scale
ASCALE = 1.0 / 8.0       # fp8 activation scale (product = 1 -> PSUM unscaled)

W16_NAMES = ["w0", "w1", "w2", "w3", "w4", "w5"]
W8_NAMES = ["m1", "m2", "m3", "g1w1", "g2a", "g2b"]
VEC_IDX = {n: i for i, n in enumerate(
    ["c0", "c1", "c2", "c3", "c4", "c5", "cm1", "cm2", "cm3", "cga",
     "g0", "b0", "g1", "b1", "g2", "b2"])}
NVEC = len(VEC_IDX)


def build_program(r=R, nb=NB, simple_ln=True):
    nblk = r // nb
    nc = bacc.Bacc("TRN2", target_bir_lowering=False, debug=False)

    # fp8 gate-path activations
    t8 = nc.dram_tensor("t8", [128, C, r], F8, kind="ExternalInput").ap()
    v8 = nc.dram_tensor("v8", [128, C, r], F8, kind="ExternalInput").ap()
    a8 = nc.dram_tensor("a8", [128, C, r], F8, kind="ExternalInput").ap()
    # fp16 main-path / residual activations
    tb = nc.dram_tensor("tb", [128, C, r], F16, kind="ExternalInput").ap()
    vb = nc.dram_tensor("vb", [128, C, r], F16, kind="ExternalInput").ap()
    ab = nc.dram_tensor("ab", [128, C, r], F16, kind="ExternalInput").ap()
    w16 = {n: nc.dram_tensor(n, [128, C, DIM], F16, kind="ExternalInput").ap()
           for n in W16_NAMES}
    w8 = {n: nc.dram_tensor(n, [128, C2, 2, DIM], F8, kind="ExternalInput").ap()
          for n in W8_NAMES}
    vecs = nc.dram_tensor("vecs", [128, NVEC, C], F32, kind="ExternalInput").ap()
    ot = nc.dram_tensor("ot", [128, C, r], F16, kind="ExternalOutput").ap()
    ov = nc.dram_tensor("ov", [128, C, r], F16, kind="ExternalOutput").ap()
    oa = nc.dram_tensor("oa", [128, C, r], F16, kind="ExternalOutput").ap()

    with tile.TileContext(nc) as tc:
        _body(tc, t8, v8, a8, tb, vb, ab, w16, w8, vecs, ot, ov, oa,
              r, nb, nblk, simple_ln)
    nc.compile()
    return nc


def _body(tc, *args):
    from contextlib import ExitStack
    with ExitStack() as _ctx:
        _body_inner(_ctx, tc, *args)


def _body_inner(_ctx, tc, t8, v8, a8, tb, vb, ab, w16d, w8d, vecs,
                ot, ov, oa, r, nb, nblk, simple_ln):
    nc = tc.nc

    cst = _ctx.enter_context(tc.tile_pool(name="cst", bufs=1))
    psum_mm = _ctx.enter_context(tc.tile_pool(name="psmm", bufs=8, space="PSUM"))
    dram = _ctx.enter_context(tc.tile_pool(name="dram", bufs=1, space="DRAM"))
    dsm = _ctx.enter_context(tc.tile_pool(name="dsm", bufs=2, space="DRAM"))

    vec_sb = cst.tile([128, NVEC, C], F32, tag="vecs")
    nc.sync.dma_start(vec_sb[:], vecs[:])
    ones16 = cst.tile([128, 1, 16], F16, tag="ocol")
    nc.vector.memset(ones16[:], 1.0)
    eps_t = cst.tile([1, 1], F32, tag="eps")
    nc.vector.memset(eps_t[:], EPS)

    # DRAM scratch for cross-sweep intermediates
    v2t_d = dram.tile([128, C, r], F16, tag="v2t_d")
    te_d = dram.tile([128, C, r], F16, tag="te_d")
    te8_d = dram.tile([128, C, r], F8, tag="te8_d")
    a2t_d = dram.tile([128, C, r], F16, tag="a2t_d")
    a2v_d = dram.tile([128, C, r], F16, tag="a2v_d")

    def vs(name, ci):
        i = VEC_IDX[name]
        return vec_sb[:, i, ci:ci + 1]

    def emit16(pairs, consume):
        """fp16 main matmuls: accumulate sum_i w_i.T @ rhs_i per out chunk."""
        total = C * len(pairs)
        for oc in range(C):
            ps = psum_mm.tile([128, nb], F32, tag="mm", bufs=6)
            idx = 0
            for w_sb, rhs in pairs:
                for kc in range(C):
                    nc.tensor.matmul(
                        ps[:], w_sb[:, kc, ts(oc, 128)], rhs[:, kc, :],
                        start=(idx == 0), stop=(idx == total - 1))
                    idx += 1
            consume(oc, ps)

    def emit8(pairs, consume):
        """fp8 DoubleRow gate matmuls (2 feature chunks per pass)."""
        total = C2 * len(pairs)
        for oc in range(C):
            ps = psum_mm.tile([128, nb], F32, tag="mm", bufs=6)
            idx = 0
            for w_sb, rhs in pairs:
                for k2 in range(C2):
                    nc.tensor.matmul(
                        ps[:], w_sb[:, k2, :, ts(oc, 128)],
                        rhs[:, 2 * k2:2 * k2 + 2, :],
                        start=(idx == 0), stop=(idx == total - 1),
                        perf_mode=PM.DoubleRow)
                    idx += 1
            consume(oc, ps)

    def evict_bias(dst, cname):
        """psum + per-feature bias -> dst (DVE)."""
        def f(oc, ps):
            nc.vector.tensor_scalar_add(dst[:, oc, :], ps[:], vs(cname, oc))
        return f

    def sum_bias(dst, cname, other):
        """(psum + bias) + other -> dst (DVE)."""
        def f(oc, ps):
            nc.vector.scalar_tensor_tensor(
                dst[:, oc, :], ps[:], vs(cname, oc), other[:, oc, :],
                OP.add, OP.add)
        return f

    def sigmoid_evict(dst, cname):
        def f(oc, ps):
            nc.scalar.activation(dst[:, oc, :], ps[:], AF.Sigmoid,
                                 bias=vs(cname, oc), scale=1.0)
        return f

    def ln_stats(pool, y, key):
        """LN stats: fp16 ones-matmul sums on y, y^2 -> rstd/shift bcast."""
        ysq = pool.tile([128, C, nb], F16, tag=f"ysq{key}", bufs=1, name="ysq")
        nc.vector.tensor_mul(ysq[:], y[:], y[:])
        s1 = psum_mm.tile([1, nb], F32, tag="st", bufs=2, name="s1")
        for ci in range(C):
            nc.tensor.matmul(s1[:], ones16[:, :, 0:1], y[:, ci, :],
                             start=(ci == 0), stop=(ci == C - 1))
        s2 = psum_mm.tile([1, nb], F32, tag="st", bufs=2, name="s2")
        for ci in range(C):
            nc.tensor.matmul(s2[:], ones16[:, :, 0:1], ysq[:, ci, :],
                             start=(ci == 0), stop=(ci == C - 1))
        negmu = pool.tile([1, nb], F32, tag="sm1", bufs=1, name="negmu")
        nc.scalar.mul(negmu[:], s1[:], -1.0 / DIM)
        t2 = pool.tile([1, nb], F32, tag="sm2", bufs=1, name="t2")
        nc.vector.tensor_mul(t2[:], negmu[:], negmu[:])          # mu^2
        t3 = pool.tile([1, nb], F32, tag="sm3", bufs=1, name="t3")
        nc.vector.scalar_tensor_tensor(t3[:], s2[:], 1.0 / DIM, t2[:],
                                       OP.mult, OP.subtract)     # var
        nc.scalar.activation(t2[:], t3[:], AF.Sqrt, bias=eps_t[:])  # sd
        nc.vector.reciprocal_approx_fast(t3[:], t2[:])           # rstd
        sh16 = pool.tile([1, nb], F16, tag="sm6", bufs=1, name="sh16")
        nc.vector.tensor_mul(sh16[:], negmu[:], t3[:])           # shift (f16)
        rs16 = pool.tile([1, nb], F16, tag="sm7", bufs=1, name="rs16")
        nc.vector.tensor_copy(rs16[:], t3[:])                    # rstd (f16)
        # broadcast rstd/shift across partitions via a DMA bounce through DRAM
        rd = dsm.tile([1, nb], F16, tag=f"rd{key}", bufs=2, name="rd")
        nc.sync.dma_start(rd[:], rs16[:])
        sh = dsm.tile([1, nb], F16, tag=f"sh{key}", bufs=2, name="sh")
        nc.sync.dma_start(sh[:], sh16[:])
        abt = pool.tile([128, 1, nb], F16, tag=f"ab{key}", bufs=2, name="ab")
        nc.sync.dma_start(abt[:, 0, :], rd[:].to_broadcast((128, nb)))
        bbt = pool.tile([128, 1, nb], F16, tag=f"bb{key}", bufs=2, name="bb")
        nc.sync.dma_start(bbt[:, 0, :], sh[:].to_broadcast((128, nb)))
        return abt, bbt

    def ln_apply(pool, st, y, out, gname, bname):
        """out = (y*rstd + shift) [* gamma + beta]."""
        abt, bbt = st
        if simple_ln:
            t = pool.tile([128, C, nb], F16, tag="lnt", bufs=1, name="t")
            nc.vector.tensor_mul(t[:], y[:], abt[:].to_broadcast((128, C, nb)))
            nc.vector.tensor_add(out[:], t[:], bbt[:].to_broadcast((128, C, nb)))
            return
        for ci in range(C):
            t = pool.tile([128, nb], F16, tag="lnt2", bufs=2, name="t")
            u = pool.tile([128, nb], F32, tag="lnu", bufs=2, name="u")
            nc.scalar.activation(u[:], bbt[:, 0, :], AF.Identity,
                                 bias=vs(bname, ci), scale=vs(gname, ci))
            nc.vector.scalar_tensor_tensor(
                t[:], y[:, ci, :], vs(gname, ci), abt[:, 0, :],
                OP.mult, OP.mult)
            nc.vector.tensor_add(out[:, ci, :], t[:], u[:])

    # rotating weight pools
    wp16 = _ctx.enter_context(tc.tile_pool(name="wp16", bufs=2))
    wp8 = _ctx.enter_context(tc.tile_pool(name="wp8", bufs=3))
    # one shared activation pool across all sweeps: shared tags rotate
    # naturally, so there is no pool-boundary barrier between sweeps.
    ap = _ctx.enter_context(tc.tile_pool(name="act", bufs=2))

    def load_w16(names):
        out = {}
        for n in names:
            out[n] = wp16.tile([128, C, DIM], F16, tag="w16", name=n)
            for kc in range(C):
                nc.sync.dma_start(out[n][:, kc, :], w16d[n][:, kc, :])
        return out

    def load_w8(names):
        out = {}
        for n in names:
            out[n] = wp8.tile([128, C2, 2, DIM], F8, tag="w8", name=n)
            for k2 in range(C2):
                nc.sync.dma_start(out[n][:, k2, :, :], w8d[n][:, k2, :, :])
        return out

    bsl = lambda b: (slice(None), slice(None), ts(b, nb))

    def stream(tag, src_d, b, dt=F16, bufs=2):
        t = ap.tile([128, C, nb], dt, tag=tag, bufs=bufs, name=tag)
        nc.sync.dma_start(t[:], src_d[bsl(b)])
        return t

    def run_sweep(mm_fn, apply_fn, gname, bname):
        hist = []
        for b in range(nblk + 2):
            if b < nblk:
                hist.append((b, mm_fn(b), None))
            if b >= 1 and b - 1 < nblk:
                bb_, y_, _ = hist[b - 1]
                hist[b - 1] = (bb_, y_, ln_stats(ap, y_, ""))
            if b >= 2:
                bb_, y_, st_ = hist[b - 2]
                apply_fn(bb_, y_, st_, gname, bname)
                hist[b - 2] = None

    # ---- Sweep 1: text stage 1 -> te_d, te8_d, v2t_d ---------------------
    w = load_w16(["w1", "w0"])
    w.update(load_w8(["m1"]))

    def s1_mm(b):
        t8s = stream("e8a", t8, b, F8, 2)
        tbs = stream("f16a", tb, b)
        vbs = stream("f16b", vb, b)
        v2t = ap.tile([128, C, nb], F16, tag="oA", bufs=1, name="v2t")
        emit16([(w["w1"], tbs)], evict_bias(v2t, "c1"))
        nc.sync.dma_start(v2t_d[bsl(b)], v2t[:])
        y = ap.tile([128, C, nb], F16, tag="y", bufs=3, name="y1")
        emit16([(w["w0"], vbs)], sum_bias(y, "c0", v2t))
        g = ap.tile([128, C, nb], F16, tag="g", bufs=1, name="g1")
        emit8([(w["m1"], t8s)], sigmoid_evict(g, "cm1"))
        nc.vector.tensor_mul(y[:], g[:], y[:])
        nc.vector.tensor_add(y[:], tbs[:], y[:])
        return y

    def s1_apply(bb_, y_, st_, gname, bname):
        te = ap.tile([128, C, nb], F16, tag="te", bufs=1, name="te")
        ln_apply(ap, st_, y_, te, gname, bname)
        nc.sync.dma_start(te_d[bsl(bb_)], te[:])
        te8t = ap.tile([128, C, nb], F8, tag="te8", bufs=2, name="te8")
        nc.scalar.mul(te8t[:], te[:], ASCALE)
        nc.sync.dma_start(te8_d[bsl(bb_)], te8t[:])

    run_sweep(s1_mm, s1_apply, "g0", "b0")

    # ---- Sweep 2: text stage 2 -> ot, a2t_d ------------------------------
    w = load_w16(["w3", "w2"])
    w.update(load_w8(["m2"]))

    def s2_mm(b):
        te8s = stream("e8a", te8_d, b, F8, 2)
        tes = stream("f16a", te_d, b)
        abs_ = stream("f16b", ab, b)
        g = ap.tile([128, C, nb], F16, tag="g", bufs=1, name="g2")
        emit8([(w["m2"], te8s)], sigmoid_evict(g, "cm2"))
        a2t = ap.tile([128, C, nb], F16, tag="oA", bufs=1, name="a2t")
        emit16([(w["w3"], tes)], evict_bias(a2t, "c3"))
        nc.sync.dma_start(a2t_d[bsl(b)], a2t[:])
        y = ap.tile([128, C, nb], F16, tag="y", bufs=3, name="y2")
        emit16([(w["w2"], abs_)], sum_bias(y, "c2", a2t))
        nc.vector.tensor_mul(y[:], g[:], y[:])
        nc.vector.tensor_add(y[:], tes[:], y[:])
        return y

    def out_apply(dst):
        def f(bb_, y_, st_, gname, bname):
            ln_apply(ap, st_, y_, y_, gname, bname)
            nc.sync.dma_start(dst[bsl(bb_)], y_[:])
        return f

    run_sweep(s2_mm, out_apply(ot), "g0", "b0")

    # ---- Sweep 3: vision -> ov, a2v_d ------------------------------------
    w = load_w16(["w5", "w4"])
    w.update(load_w8(["m3", "g1w1"]))

    def s3_mm(b):
        v8s = stream("e8a", v8, b, F8, 2)
        t8s = stream("e8b", t8, b, F8, 1)
        vbs = stream("f16a", vb, b)
        abs_ = stream("f16b", ab, b)
        v2ts = stream("f16c", v2t_d, b)
        g = ap.tile([128, C, nb], F16, tag="g", bufs=1, name="gv")
        emit8([(w["m3"], v8s), (w["g1w1"], t8s)], sigmoid_evict(g, "cm3"))
        a2v = ap.tile([128, C, nb], F16, tag="oA", bufs=1, name="a2v")
        emit16([(w["w5"], vbs)], evict_bias(a2v, "c5"))
        nc.sync.dma_start(a2v_d[bsl(b)], a2v[:])
        y = ap.tile([128, C, nb], F16, tag="y", bufs=3, name="yv")
        emit16([(w["w4"], abs_)], sum_bias(y, "c4", a2v))
        nc.vector.tensor_add(y[:], y[:], v2ts[:])
        nc.vector.tensor_mul(y[:], g[:], y[:])
        nc.vector.tensor_add(y[:], vbs[:], y[:])
        return y

    run_sweep(s3_mm, out_apply(ov), "g1", "b1")

    # ---- Sweep 4: audio -> oa --------------------------------------------
    w = load_w8(["g2a", "g2b"])

    # S4's gate unit depends on sa8 = f8(a2t + a2v); prep one block ahead so
    # the DVE add + ACT copy overlap the previous block's matmuls.
    s4_prep = {}

    def s4_do_prep(b):
        a8s = stream("e8a", a8, b, F8, 2)
        abs_ = stream("f16a", ab, b)
        a2ts = stream("f16b", a2t_d, b)
        a2vs = stream("f16c", a2v_d, b)
        sa = ap.tile([128, C, nb], F16, tag="oA", bufs=1, name="sa")
        nc.vector.tensor_add(sa[:], a2ts[:], a2vs[:])
        sa8 = ap.tile([128, C, nb], F8, tag="te8", bufs=2, name="sa8")
        nc.scalar.mul(sa8[:], sa[:], ASCALE)
        s4_prep[b] = (a8s, abs_, sa, sa8)

    def s4_mm(b):
        if b == 0:
            s4_do_prep(0)
        a8s, abs_, sa, sa8 = s4_prep.pop(b)
        if b + 1 < nblk:
            s4_do_prep(b + 1)
        g = ap.tile([128, C, nb], F16, tag="g", bufs=1, name="ga")
        emit8([(w["g2a"], a8s), (w["g2b"], sa8)], sigmoid_evict(g, "cga"))
        y = ap.tile([128, C, nb], F16, tag="y", bufs=3, name="ya")
        nc.vector.tensor_mul(y[:], g[:], sa[:])
        nc.vector.tensor_add(y[:], abs_[:], y[:])
        return y

    run_sweep(s4_mm, out_apply(oa), "g2", "b2")


# ---------------------------------------------------------------------------
# Host side
# ---------------------------------------------------------------------------

def _to_dev_act8(x):
    """[rows, 1024] fp32 -> [128, C, rows] fp8e4 (x * ASCALE)."""
    r = x.shape[0]
    xt = np.ascontiguousarray(x.T.reshape(C, 128, r).transpose(1, 0, 2))
    return np.clip(xt * ASCALE, -240, 240).astype(F8NP)


def _to_dev_act16(x):
    """[rows, 1024] fp32 -> [128, C, rows] fp16."""
    r = x.shape[0]
    return np.ascontiguousarray(
        x.T.reshape(C, 128, r).transpose(1, 0, 2)).astype(np.float16)


def _to_dev_w16(m):
    """W [1024(out), 1024(in)] -> lhsT [128, C(kc), 1024(out)] fp16."""
    return np.ascontiguousarray(
        m.reshape(DIM, C, 128).transpose(2, 1, 0)).astype(np.float16)


def _to_dev_w8(m):
    """W [1024(out), 1024(in)] -> DoubleRow lhsT [128, C2, 2, 1024] fp8."""
    mq = np.clip(m * WSCALE, -240, 240)
    arr = np.ascontiguousarray(mq.T.reshape(C2, 2, 128, DIM).transpose(2, 0, 1, 3))
    return arr.astype(F8NP)


def _from_dev_out16(o):
    """[128, C, rows] fp16 -> [rows, 1024] fp32."""
    r = o.shape[2]
    return np.ascontiguousarray(
        o.astype(np.float32).transpose(1, 0, 2).reshape(DIM, r).T)


_PROG = {}


def _get_prog(simple_ln):
    if simple_ln not in _PROG:
        _PROG[simple_ln] = build_program(simple_ln=simple_ln)
    return _PROG[simple_ln]


def fold_weights(mha_w_in, mha_b_in, mha_w_out, mha_b_out, gate_w, gate_b):
    W, cvec = [], []
    for i in range(6):
        w_v = mha_w_in[i][2 * DIM:3 * DIM]
        b_v = mha_b_in[i][2 * DIM:3 * DIM]
        W.append(mha_w_out[i] @ w_v)
        cvec.append(mha_w_out[i] @ b_v + mha_b_out[i])
    Ga = [gate_w[j][:, :DIM] for j in range(3)]
    Gb = [gate_w[j][:, DIM:] for j in range(3)]
    M1 = Ga[0] + Gb[0] @ W[1]
    cM1 = gate_b[0] + Gb[0] @ cvec[1]
    M2 = Ga[0] + Gb[0] @ W[3]
    cM2 = gate_b[0] + Gb[0] @ cvec[3]
    M3 = Ga[1] + Gb[1] @ W[5]
    cM3 = gate_b[1] + Gb[1] @ cvec[5] + Gb[1] @ cvec[1]
    G1W1 = Gb[1] @ W[1]
    w16 = {"w0": W[0], "w1": W[1], "w2": W[2], "w3": W[3], "w4": W[4],
           "w5": W[5]}
    w8 = {"m1": M1, "m2": M2, "m3": M3, "g1w1": G1W1, "g2a": Ga[2],
          "g2b": Gb[2]}
    return w16, w8, cvec, (cM1, cM2, cM3)


LAST_EXEC_TIME_NS = None


def timed_run(inputs):
    """Re-run the kernel with NTFF tracing; returns HW exec time in ns."""
    kernel(**inputs, _trace=True)
    return LAST_EXEC_TIME_NS


def kernel(text, vision, audio, mha_w_in, mha_b_in, mha_w_out, mha_b_out,
           gate_w, gate_b, ln_scale, ln_bias, _trace=False):
    f32 = lambda a: np.asarray(a, dtype=np.float32)
    text, vision, audio = f32(text), f32(vision), f32(audio)
    mha_w_in, mha_b_in = f32(mha_w_in), f32(mha_b_in)
    mha_w_out, mha_b_out = f32(mha_w_out), f32(mha_b_out)
    gate_w, gate_b = f32(gate_w), f32(gate_b)
    ln_scale, ln_bias = f32(ln_scale), f32(ln_bias)

    simple_ln = bool(np.all(ln_scale == 1.0) and np.all(ln_bias == 0.0))
    nc = _get_prog(simple_ln)

    w16, w8, cvec, (cM1, cM2, cM3) = fold_weights(
        mha_w_in, mha_b_in, mha_w_out, mha_b_out, gate_w, gate_b)
    wdev = {n: _to_dev_w16(m) for n, m in w16.items()}
    wdev.update({n: _to_dev_w8(m) for n, m in w8.items()})

    V = np.zeros((NVEC, DIM), np.float32)
    for i in range(6):
        V[VEC_IDX[f"c{i}"]] = cvec[i]
    V[VEC_IDX["cm1"]], V[VEC_IDX["cm2"]], V[VEC_IDX["cm3"]] = cM1, cM2, cM3
    V[VEC_IDX["cga"]] = gate_b[2]
    for j in range(3):
        V[VEC_IDX[f"g{j}"]] = ln_scale[j]
        V[VEC_IDX[f"b{j}"]] = ln_bias[j]
    vecs_dev = np.ascontiguousarray(
        V.reshape(NVEC, C, 128).transpose(2, 0, 1)).astype(np.float32)

    in_maps = []
    for cid in range(NCORES):
        sl = slice(cid * R, (cid + 1) * R)
        in_maps.append({
            "t8": _to_dev_act8(text[sl]),
            "v8": _to_dev_act8(vision[sl]),
            "a8": _to_dev_act8(audio[sl]),
            "tb": _to_dev_act16(text[sl]),
            "vb": _to_dev_act16(vision[sl]),
            "ab": _to_dev_act16(audio[sl]),
            "vecs": vecs_dev,
            **wdev,
        })

    # The device occasionally throws a transient NRT_EXEC_UNIT_UNRECOVERABLE
    # on the first execute; retry a couple of times before giving up.
    last_err = None
    for attempt in range(3):
        try:
            res = bass_utils.run_bass_kernel_spmd(
                nc, in_maps, core_ids=list(range(NCORES)), trace=_trace)
            break
        except Exception as e:
            last_err = e
            import time as _time
            _time.sleep(5)
    else:
        raise last_err
    if _trace:
        global LAST_EXEC_TIME_NS
        LAST_EXEC_TIME_NS = res.exec_time_ns
        if res.instructions_and_trace:
            print("trace:", res.instructions_and_trace[1])

    outs = {k: np.empty((BATCH, DIM), np.float32) for k in ("ot", "ov", "oa")}
    for cid in range(NCORES):
        sl = slice(cid * R, (cid + 1) * R)
        for k in outs:
            outs[k][sl] = _from_dev_out16(res.results[cid][k])
    return (outs["ot"], outs["ov"], outs["oa"])


# revision 20
# speedup vs baseline: 1.0140x; 1.0140x over previous
"""Trainium2 Bass kernel for BidirectionalCrossModalAttention (seq_len=1).

Math: with a single key, softmax == 1 exactly, so each MHA block reduces to
    mha(q, kv) = kv @ (w_out @ w_v).T + (w_out @ b_v + b_out)
i.e. one 1024x1024 matmul.  Gate matmuls partially fold into the same form;
the vision gate's v2t term folds onto text:  Gb1 @ v2t = (Gb1 @ W1) @ text + c.
Total device work: 12 x [1024x1024] matmuls per row + 4 sigmoid gates +
4 layernorms.

Precision split (driven by the 2e-2 gate):
  - 6 "main" matmuls (w0..w5, outputs feed residual sums directly): fp16
    weights x fp16 activations, normal PE mode.
  - 6 "gate" matmuls (sigmoid args, error damped by sigmoid slope) +
    LN-stats ones-matmuls: fp8e4 with DoubleRow (2x contraction per pass).
    fp8 weights are scaled 8x, fp8 activations 1/8x, so PSUM is unscaled.
  - Elementwise chain + residual streams: fp16. LN stats via PE ones-matmuls
    on fp8 copies; rstd via reciprocal_approx_fast (DVE).

Engine split: PE matmuls; DVE evictions (bias add / sum) + y-chain + stats;
ACT only Sigmoid + Sqrt (avoids activation-table thrash); GpSimd makes the
fp8 stat copies (yb, y^2) and fp8 downscale copies.

Layout: transposed — activations [feature(->128 partitions, 8 chunks),
rows(->free)].  Row blocks of 512 (one full PSUM bank per matmul output).
LayerNorm row-stats broadcast across partitions via a DMA bounce thru DRAM.

Sharding: pure data parallelism — batch 32768 = 8 x 4096 across cores.
"""

import numpy as np
import ml_dtypes

import concourse.bass as bass
import concourse.tile as tile
from concourse import bacc, mybir
from concourse import bass_utils
from concourse.bass import ts

F32 = mybir.dt.float32
F16 = mybir.dt.float16
F8 = mybir.dt.float8e4
AF = mybir.ActivationFunctionType
OP = mybir.AluOpType
PM = mybir.MatmulPerfMode

F8NP = ml_dtypes.float8_e4m3

DIM = 1024
BATCH = 32768
NCORES = 8
R = BATCH // NCORES      # rows per core
C = DIM // 128           # feature chunks
C2 = C // 2              # feature chunk pairs (DoubleRow)
EPS = 1e-5

NB = 512                 # rows per block (one PSUM bank of fp32)
NBLK = R // NB

WSCALE = 8.0             # fp8 weight scale
ASCALE = 1.0 / 8.0       # fp8 activation scale (product = 1 -> PSUM unscaled)

W16_NAMES = ["w0", "w1", "w2", "w3", "w4", "w5"]
W8_NAMES = ["m1", "m2", "m3", "g1w1", "g2a", "g2b"]
VEC_IDX = {n: i for i, n in enumerate(
    ["c0", "c1", "c2", "c3", "c4", "c5", "cm1", "cm2", "cm3", "cga",
     "g0", "b0", "g1", "b1", "g2", "b2"])}
NVEC = len(VEC_IDX)


def build_program(r=R, nb=NB, simple_ln=True):
    nblk = r // nb
    nc = bacc.Bacc("TRN2", target_bir_lowering=False, debug=False)

    # fp8 gate-path activations
    t8 = nc.dram_tensor("t8", [128, C, r], F8, kind="ExternalInput").ap()
    v8 = nc.dram_tensor("v8", [128, C, r], F8, kind="ExternalInput").ap()
    a8 = nc.dram_tensor("a8", [128, C, r], F8, kind="ExternalInput").ap()
    # fp16 main-path / residual activations
    tb = nc.dram_tensor("tb", [128, C, r], F16, kind="ExternalInput").ap()
    vb = nc.dram_tensor("vb", [128, C, r], F16, kind="ExternalInput").ap()
    ab = nc.dram_tensor("ab", [128, C, r], F16, kind="ExternalInput").ap()
    w16 = {n: nc.dram_tensor(n, [128, C, DIM], F16, kind="ExternalInput").ap()
           for n in W16_NAMES}
    w8 = {n: nc.dram_tensor(n, [128, C2, 2, DIM], F8, kind="ExternalInput").ap()
          for n in W8_NAMES}
    vecs = nc.dram_tensor("vecs", [128, NVEC, C], F32, kind="ExternalInput").ap()
    ot = nc.dram_tensor("ot", [128, C, r], F16, kind="ExternalOutput").ap()
    ov = nc.dram_tensor("ov", [128, C, r], F16, kind="ExternalOutput").ap()
    oa = nc.dram_tensor("oa", [128, C, r], F16, kind="ExternalOutput").ap()

    with tile.TileContext(nc) as tc:
        _body(tc, t8, v8, a8, tb, vb, ab, w16, w8, vecs, ot, ov, oa,
              r, nb, nblk, simple_ln)
    nc.compile()
    return nc


def _body(tc, *args):
    from contextlib import ExitStack
    with ExitStack() as _ctx:
        _body_inner(_ctx, tc, *args)


def _body_inner(_ctx, tc, t8, v8, a8, tb, vb, ab, w16d, w8d, vecs,
                ot, ov, oa, r, nb, nblk, simple_ln):
    nc = tc.nc

    cst = _ctx.enter_context(tc.tile_pool(name="cst", bufs=1))
    psum_mm = _ctx.enter_context(tc.tile_pool(name="psmm", bufs=8, space="PSUM"))
    dram = _ctx.enter_context(tc.tile_pool(name="dram", bufs=1, space="DRAM"))
    dsm = _ctx.enter_context(tc.tile_pool(name="dsm", bufs=2, space="DRAM"))

    vec_sb = cst.tile([128, NVEC, C], F32, tag="vecs")
    nc.sync.dma_start(vec_sb[:], vecs[:])
    ones16 = cst.tile([128, 1, 16], F16, tag="ocol")
    nc.vector.memset(ones16[:], 1.0)
    eps_t = cst.tile([1, 1], F32, tag="eps")
    nc.vector.memset(eps_t[:], EPS)

    # DRAM scratch for cross-sweep intermediates
    v2t_d = dram.tile([128, C, r], F16, tag="v2t_d")
    te_d = dram.tile([128, C, r], F16, tag="te_d")
    te8_d = dram.tile([128, C, r], F8, tag="te8_d")
    a2t_d = dram.tile([128, C, r], F16, tag="a2t_d")
    a2v_d = dram.tile([128, C, r], F16, tag="a2v_d")

    def vs(name, ci):
        i = VEC_IDX[name]
        return vec_sb[:, i, ci:ci + 1]

    def emit16(pairs, consume):
        """fp16 main matmuls: accumulate sum_i w_i.T @ rhs_i per out chunk."""
        total = C * len(pairs)
        for oc in range(C):
            ps = psum_mm.tile([128, nb], F32, tag="mm", bufs=5)
            idx = 0
            for w_sb, rhs in pairs:
                for kc in range(C):
                    nc.tensor.matmul(
                        ps[:], w_sb[:, kc, ts(oc, 128)], rhs[:, kc, :],
                        start=(idx == 0), stop=(idx == total - 1))
                    idx += 1
            consume(oc, ps)

    def emit8(pairs, consume):
        """fp8 DoubleRow gate matmuls (2 feature chunks per pass)."""
        total = C2 * len(pairs)
        for oc in range(C):
            ps = psum_mm.tile([128, nb], F32, tag="mm", bufs=5)
            idx = 0
            for w_sb, rhs in pairs:
                for k2 in range(C2):
                    nc.tensor.matmul(
                        ps[:], w_sb[:, k2, :, ts(oc, 128)],
                        rhs[:, 2 * k2:2 * k2 + 2, :],
                        start=(idx == 0), stop=(idx == total - 1),
                        perf_mode=PM.DoubleRow)
                    idx += 1
            consume(oc, ps)

    def evict_bias(dst, cname):
        """psum + per-feature bias -> dst (DVE)."""
        def f(oc, ps):
            nc.vector.tensor_scalar_add(dst[:, oc, :], ps[:], vs(cname, oc))
        return f

    def sum_bias(dst, cname, other):
        """(psum + bias) + other -> dst (DVE)."""
        def f(oc, ps):
            nc.vector.scalar_tensor_tensor(
                dst[:, oc, :], ps[:], vs(cname, oc), other[:, oc, :],
                OP.add, OP.add)
        return f

    def sigmoid_evict(dst, cname):
        def f(oc, ps):
            nc.scalar.activation(dst[:, oc, :], ps[:], AF.Sigmoid,
                                 bias=vs(cname, oc), scale=1.0)
        return f

    def ln_stats(pool, y, key):
        """LN stats: fp16 ones-matmul sums on y, y^2 -> rstd/shift bcast."""
        ysq = pool.tile([128, C, nb], F16, tag=f"ysq{key}", bufs=1, name="ysq")
        nc.vector.tensor_mul(ysq[:], y[:], y[:])
        s1 = psum_mm.tile([1, nb], F32, tag="st", bufs=3, name="s1")
        for ci in range(C):
            nc.tensor.matmul(s1[:], ones16[:, :, 0:1], y[:, ci, :],
                             start=(ci == 0), stop=(ci == C - 1))
        s2 = psum_mm.tile([1, nb], F32, tag="st", bufs=3, name="s2")
        for ci in range(C):
            nc.tensor.matmul(s2[:], ones16[:, :, 0:1], ysq[:, ci, :],
                             start=(ci == 0), stop=(ci == C - 1))
        negmu = pool.tile([1, nb], F32, tag="sm1", bufs=1, name="negmu")
        nc.scalar.mul(negmu[:], s1[:], -1.0 / DIM)
        t2 = pool.tile([1, nb], F32, tag="sm2", bufs=1, name="t2")
        nc.vector.tensor_mul(t2[:], negmu[:], negmu[:])          # mu^2
        t3 = pool.tile([1, nb], F32, tag="sm3", bufs=1, name="t3")
        nc.vector.scalar_tensor_tensor(t3[:], s2[:], 1.0 / DIM, t2[:],
                                       OP.mult, OP.subtract)     # var
        nc.scalar.activation(t2[:], t3[:], AF.Sqrt, bias=eps_t[:])  # sd
        nc.vector.reciprocal_approx_fast(t3[:], t2[:])           # rstd
        sh16 = pool.tile([1, nb], F16, tag="sm6", bufs=1, name="sh16")
        nc.vector.tensor_mul(sh16[:], negmu[:], t3[:])           # shift (f16)
        rs16 = pool.tile([1, nb], F16, tag="sm7", bufs=1, name="rs16")
        nc.vector.tensor_copy(rs16[:], t3[:])                    # rstd (f16)
        # broadcast rstd/shift across partitions via a DMA bounce through DRAM
        rd = dsm.tile([1, nb], F16, tag=f"rd{key}", bufs=2, name="rd")
        nc.sync.dma_start(rd[:], rs16[:])
        sh = dsm.tile([1, nb], F16, tag=f"sh{key}", bufs=2, name="sh")
        nc.sync.dma_start(sh[:], sh16[:])
        abt = pool.tile([128, 1, nb], F16, tag=f"ab{key}", bufs=2, name="ab")
        nc.sync.dma_start(abt[:, 0, :], rd[:].to_broadcast((128, nb)))
        bbt = pool.tile([128, 1, nb], F16, tag=f"bb{key}", bufs=2, name="bb")
        nc.sync.dma_start(bbt[:, 0, :], sh[:].to_broadcast((128, nb)))
        return abt, bbt

    def ln_apply(pool, st, y, out, gname, bname):
        """out = (y*rstd + shift) [* gamma + beta]."""
        abt, bbt = st
        if simple_ln:
            t = pool.tile([128, C, nb], F16, tag="lnt", bufs=1, name="t")
            nc.vector.tensor_mul(t[:], y[:], abt[:].to_broadcast((128, C, nb)))
            nc.vector.tensor_add(out[:], t[:], bbt[:].to_broadcast((128, C, nb)))
            return
        for ci in range(C):
            t = pool.tile([128, nb], F16, tag="lnt2", bufs=2, name="t")
            u = pool.tile([128, nb], F32, tag="lnu", bufs=2, name="u")
            nc.scalar.activation(u[:], bbt[:, 0, :], AF.Identity,
                                 bias=vs(bname, ci), scale=vs(gname, ci))
            nc.vector.scalar_tensor_tensor(
                t[:], y[:, ci, :], vs(gname, ci), abt[:, 0, :],
                OP.mult, OP.mult)
            nc.vector.tensor_add(out[:, ci, :], t[:], u[:])

    # rotating weight pools
    wp16 = _ctx.enter_context(tc.tile_pool(name="wp16", bufs=2))
    wp8 = _ctx.enter_context(tc.tile_pool(name="wp8", bufs=3))
    # one shared activation pool across all sweeps: shared tags rotate
    # naturally, so there is no pool-boundary barrier between sweeps.
    ap = _ctx.enter_context(tc.tile_pool(name="act", bufs=2))

    def load_w16(names):
        out = {}
        for n in names:
            out[n] = wp16.tile([128, C, DIM], F16, tag="w16", name=n)
            for kc in range(C):
                nc.sync.dma_start(out[n][:, kc, :], w16d[n][:, kc, :])
        return out

    def load_w8(names):
        out = {}
        for n in names:
            out[n] = wp8.tile([128, C2, 2, DIM], F8, tag="w8", name=n)
            for k2 in range(C2):
                nc.sync.dma_start(out[n][:, k2, :, :], w8d[n][:, k2, :, :])
        return out

    bsl = lambda b: (slice(None), slice(None), ts(b, nb))

    def stream(tag, src_d, b, dt=F16, bufs=2):
        t = ap.tile([128, C, nb], dt, tag=tag, bufs=bufs, name=tag)
        nc.sync.dma_start(t[:], src_d[bsl(b)])
        return t

    def run_sweep(mm_fn, apply_fn, gname, bname):
        hist = []
        for b in range(nblk + 2):
            if b < nblk:
                hist.append((b, mm_fn(b), None))
            if b >= 1 and b - 1 < nblk:
                bb_, y_, _ = hist[b - 1]
                hist[b - 1] = (bb_, y_, ln_stats(ap, y_, ""))
            if b >= 2:
                bb_, y_, st_ = hist[b - 2]
                apply_fn(bb_, y_, st_, gname, bname)
                hist[b - 2] = None

    # ---- Sweep 1: text stage 1 -> te_d, te8_d, v2t_d ---------------------
    w = load_w16(["w1", "w0"])
    w.update(load_w8(["m1"]))

    def s1_mm(b):
        t8s = stream("e8a", t8, b, F8, 2)
        tbs = stream("f16a", tb, b)
        vbs = stream("f16b", vb, b)
        g = ap.tile([128, C, nb], F16, tag="g", bufs=1, name="g1")
        emit8([(w["m1"], t8s)], sigmoid_evict(g, "cm1"))
        v2t = ap.tile([128, C, nb], F16, tag="oA", bufs=1, name="v2t")
        emit16([(w["w1"], tbs)], evict_bias(v2t, "c1"))
        nc.sync.dma_start(v2t_d[bsl(b)], v2t[:])
        y = ap.tile([128, C, nb], F16, tag="y", bufs=3, name="y1")
        emit16([(w["w0"], vbs)], sum_bias(y, "c0", v2t))
        nc.vector.tensor_mul(y[:], g[:], y[:])
        nc.vector.tensor_add(y[:], tbs[:], y[:])
        return y

    def s1_apply(bb_, y_, st_, gname, bname):
        te = ap.tile([128, C, nb], F16, tag="te", bufs=1, name="te")
        ln_apply(ap, st_, y_, te, gname, bname)
        nc.sync.dma_start(te_d[bsl(bb_)], te[:])
        te8t = ap.tile([128, C, nb], F8, tag="te8", bufs=2, name="te8")
        nc.scalar.mul(te8t[:], te[:], ASCALE)
        nc.sync.dma_start(te8_d[bsl(bb_)], te8t[:])

    run_sweep(s1_mm, s1_apply, "g0", "b0")

    # ---- Sweep 2: text stage 2 -> ot, a2t_d ------------------------------
    w = load_w16(["w3", "w2"])
    w.update(load_w8(["m2"]))

    def s2_mm(b):
        te8s = stream("e8a", te8_d, b, F8, 2)
        tes = stream("f16a", te_d, b)
        abs_ = stream("f16b", ab, b)
        g = ap.tile([128, C, nb], F16, tag="g", bufs=1, name="g2")
        emit8([(w["m2"], te8s)], sigmoid_evict(g, "cm2"))
        a2t = ap.tile([128, C, nb], F16, tag="oA", bufs=1, name="a2t")
        emit16([(w["w3"], tes)], evict_bias(a2t, "c3"))
        nc.sync.dma_start(a2t_d[bsl(b)], a2t[:])
        y = ap.tile([128, C, nb], F16, tag="y", bufs=3, name="y2")
        emit16([(w["w2"], abs_)], sum_bias(y, "c2", a2t))
        nc.vector.tensor_mul(y[:], g[:], y[:])
        nc.vector.tensor_add(y[:], tes[:], y[:])
        return y

    def out_apply(dst):
        def f(bb_, y_, st_, gname, bname):
            ln_apply(ap, st_, y_, y_, gname, bname)
            nc.sync.dma_start(dst[bsl(bb_)], y_[:])
        return f

    run_sweep(s2_mm, out_apply(ot), "g0", "b0")

    # ---- Sweep 3: vision -> ov, a2v_d ------------------------------------
    w = load_w16(["w5", "w4"])
    w.update(load_w8(["m3", "g1w1"]))

    def s3_mm(b):
        v8s = stream("e8a", v8, b, F8, 2)
        t8s = stream("e8b", t8, b, F8, 1)
        vbs = stream("f16a", vb, b)
        abs_ = stream("f16b", ab, b)
        v2ts = stream("f16c", v2t_d, b)
        g = ap.tile([128, C, nb], F16, tag="g", bufs=1, name="gv")
        emit8([(w["m3"], v8s), (w["g1w1"], t8s)], sigmoid_evict(g, "cm3"))
        a2v = ap.tile([128, C, nb], F16, tag="oA", bufs=1, name="a2v")
        emit16([(w["w5"], vbs)], evict_bias(a2v, "c5"))
        nc.sync.dma_start(a2v_d[bsl(b)], a2v[:])
        y = ap.tile([128, C, nb], F16, tag="y", bufs=3, name="yv")
        emit16([(w["w4"], abs_)], sum_bias(y, "c4", a2v))
        nc.vector.tensor_add(y[:], y[:], v2ts[:])
        nc.vector.tensor_mul(y[:], g[:], y[:])
        nc.vector.tensor_add(y[:], vbs[:], y[:])
        return y

    run_sweep(s3_mm, out_apply(ov), "g1", "b1")

    # ---- Sweep 4: audio -> oa --------------------------------------------
    w = load_w8(["g2a", "g2b"])

    # S4's gate unit depends on sa8 = f8(a2t + a2v); prep one block ahead so
    # the DVE add + ACT copy overlap the previous block's matmuls.
    s4_prep = {}

    def s4_do_prep(b):
        a8s = stream("e8a", a8, b, F8, 2)
        abs_ = stream("f16a", ab, b)
        a2ts = stream("f16b", a2t_d, b)
        a2vs = stream("f16c", a2v_d, b)
        sa = ap.tile([128, C, nb], F16, tag="oA", bufs=1, name="sa")
        nc.vector.tensor_add(sa[:], a2ts[:], a2vs[:])
        sa8 = ap.tile([128, C, nb], F8, tag="te8", bufs=2, name="sa8")
        nc.scalar.mul(sa8[:], sa[:], ASCALE)
        s4_prep[b] = (a8s, abs_, sa, sa8)

    def s4_mm(b):
        if b == 0:
            s4_do_prep(0)
        a8s, abs_, sa, sa8 = s4_prep.pop(b)
        if b + 1 < nblk:
            s4_do_prep(b + 1)
        g = ap.tile([128, C, nb], F16, tag="g", bufs=1, name="ga")
        emit8([(w["g2a"], a8s), (w["g2b"], sa8)], sigmoid_evict(g, "cga"))
        y = ap.tile([128, C, nb], F16, tag="y", bufs=3, name="ya")
        nc.vector.tensor_mul(y[:], g[:], sa[:])
        nc.vector.tensor_add(y[:], abs_[:], y[:])
        return y

    run_sweep(s4_mm, out_apply(oa), "g2", "b2")


# ---------------------------------------------------------------------------
# Host side
# ---------------------------------------------------------------------------

def _to_dev_act8(x):
    """[rows, 1024] fp32 -> [128, C, rows] fp8e4 (x * ASCALE)."""
    r = x.shape[0]
    xt = np.ascontiguousarray(x.T.reshape(C, 128, r).transpose(1, 0, 2))
    return np.clip(xt * ASCALE, -240, 240).astype(F8NP)


def _to_dev_act16(x):
    """[rows, 1024] fp32 -> [128, C, rows] fp16."""
    r = x.shape[0]
    return np.ascontiguousarray(
        x.T.reshape(C, 128, r).transpose(1, 0, 2)).astype(np.float16)


def _to_dev_w16(m):
    """W [1024(out), 1024(in)] -> lhsT [128, C(kc), 1024(out)] fp16."""
    return np.ascontiguousarray(
        m.reshape(DIM, C, 128).transpose(2, 1, 0)).astype(np.float16)


def _to_dev_w8(m):
    """W [1024(out), 1024(in)] -> DoubleRow lhsT [128, C2, 2, 1024] fp8."""
    mq = np.clip(m * WSCALE, -240, 240)
    arr = np.ascontiguousarray(mq.T.reshape(C2, 2, 128, DIM).transpose(2, 0, 1, 3))
    return arr.astype(F8NP)


def _from_dev_out16(o):
    """[128, C, rows] fp16 -> [rows, 1024] fp32."""
    r = o.shape[2]
    return np.ascontiguousarray(
        o.astype(np.float32).transpose(1, 0, 2).reshape(DIM, r).T)


_PROG = {}


def _get_prog(simple_ln):
    if simple_ln not in _PROG:
        _PROG[simple_ln] = build_program(simple_ln=simple_ln)
    return _PROG[simple_ln]


def fold_weights(mha_w_in, mha_b_in, mha_w_out, mha_b_out, gate_w, gate_b):
    W, cvec = [], []
    for i in range(6):
        w_v = mha_w_in[i][2 * DIM:3 * DIM]
        b_v = mha_b_in[i][2 * DIM:3 * DIM]
        W.append(mha_w_out[i] @ w_v)
        cvec.append(mha_w_out[i] @ b_v + mha_b_out[i])
    Ga = [gate_w[j][:, :DIM] for j in range(3)]
    Gb = [gate_w[j][:, DIM:] for j in range(3)]
    M1 = Ga[0] + Gb[0] @ W[1]
    cM1 = gate_b[0] + Gb[0] @ cvec[1]
    M2 = Ga[0] + Gb[0] @ W[3]
    cM2 = gate_b[0] + Gb[0] @ cvec[3]
    M3 = Ga[1] + Gb[1] @ W[5]
    cM3 = gate_b[1] + Gb[1] @ cvec[5] + Gb[1] @ cvec[1]
    G1W1 = Gb[1] @ W[1]
    w16 = {"w0": W[0], "w1": W[1], "w2": W[2], "w3": W[3], "w4": W[4],
           "w5": W[5]}
    w8 = {"m1": M1, "m2": M2, "m3": M3, "g1w1": G1W1, "g2a": Ga[2],
          "g2b": Gb[2]}
    return w16, w8, cvec, (cM1, cM2, cM3)


LAST_EXEC_TIME_NS = None


def timed_run(inputs):
    """Re-run the kernel with NTFF tracing; returns HW exec time in ns."""
    kernel(**inputs, _trace=True)
    return LAST_EXEC_TIME_NS


def kernel(text, vision, audio, mha_w_in, mha_b_in, mha_w_out, mha_b_out,
           gate_w, gate_b, ln_scale, ln_bias, _trace=False):
    f32 = lambda a: np.asarray(a, dtype=np.float32)
    text, vision, audio = f32(text), f32(vision), f32(audio)
    mha_w_in, mha_b_in = f32(mha_w_in), f32(mha_b_in)
    mha_w_out, mha_b_out = f32(mha_w_out), f32(mha_b_out)
    gate_w, gate_b = f32(gate_w), f32(gate_b)
    ln_scale, ln_bias = f32(ln_scale), f32(ln_bias)

    simple_ln = bool(np.all(ln_scale == 1.0) and np.all(ln_bias == 0.0))
    nc = _get_prog(simple_ln)

    w16, w8, cvec, (cM1, cM2, cM3) = fold_weights(
        mha_w_in, mha_b_in, mha_w_out, mha_b_out, gate_w, gate_b)
    wdev = {n: _to_dev_w16(m) for n, m in w16.items()}
    wdev.update({n: _to_dev_w8(m) for n, m in w8.items()})

    V = np.zeros((NVEC, DIM), np.float32)
    for i in range(6):
        V[VEC_IDX[f"c{i}"]] = cvec[i]
    V[VEC_IDX["cm1"]], V[VEC_IDX["cm2"]], V[VEC_IDX["cm3"]] = cM1, cM2, cM3
    V[VEC_IDX["cga"]] = gate_b[2]
    for j in range(3):
        V[VEC_IDX[f"g{j}"]] = ln_scale[j]
        V[VEC_IDX[f"b{j}"]] = ln_bias[j]
    vecs_dev = np.ascontiguousarray(
        V.reshape(NVEC, C, 128).transpose(2, 0, 1)).astype(np.float32)

    in_maps = []
    for cid in range(NCORES):
        sl = slice(cid * R, (cid + 1) * R)
        in_maps.append({
            "t8": _to_dev_act8(text[sl]),
            "v8": _to_dev_act8(vision[sl]),
            "a8": _to_dev_act8(audio[sl]),
            "tb": _to_dev_act16(text[sl]),
            "vb": _to_dev_act16(vision[sl]),
            "ab": _to_dev_act16(audio[sl]),
            "vecs": vecs_dev,
            **wdev,
        })

    # The device occasionally throws a transient NRT_EXEC_UNIT_UNRECOVERABLE
    # on the first execute; retry a couple of times before giving up.
    last_err = None
    for attempt in range(3):
        try:
            res = bass_utils.run_bass_kernel_spmd(
                nc, in_maps, core_ids=list(range(NCORES)), trace=_trace)
            break
        except Exception as e:
            last_err = e
            import time as _time
            _time.sleep(5)
    else:
        raise last_err
    if _trace:
        global LAST_EXEC_TIME_NS
        LAST_EXEC_TIME_NS = res.exec_time_ns
        if res.instructions_and_trace:
            print("trace:", res.instructions_and_trace[1])

    outs = {k: np.empty((BATCH, DIM), np.float32) for k in ("ot", "ov", "oa")}
    for cid in range(NCORES):
        sl = slice(cid * R, (cid + 1) * R)
        for k in outs:
            outs[k][sl] = _from_dev_out16(res.results[cid][k])
    return (outs["ot"], outs["ov"], outs["oa"])
